# revision 12
# baseline (speedup 1.0000x reference)
"""CFConvCluster Trainium2 kernel (8 NeuronCores, SPMD, no collectives).

Strategy
--------
The reference computes, per edge e:  msg_e = mask_e * new_node[src_e] * MLP(rbf_e)
and scatter-sums msg into dst nodes.  Exact algebraic restructurings:

1. Masked edges contribute exactly zero -> dropped up front (E: 1.6M -> ~449k).
2. Nodes are relabeled (host permutation) into 784 balanced 128-node
   "windows"; edges grouped by the window of their dst.  The segment-sum
   for a window is a matmul with a one-hot selection matrix
   S_T[e, n] = (dst_e == slot n), accumulated over the window's edge
   tiles in PSUM.  Output ranges are disjoint across cores -> no
   all-reduce; the host concatenates and un-permutes rows.
3. b2 folds via linearity: sum S*(h2+b2)*g = sum S*(h2*g) + b2*sum S*g,
   computed as twin accumulated matmuls sharing one PSUM tile, combined
   by a single fused DVE op.

Windows are processed in PAIRS: the edge MLP for window pair (wA, wB)
runs on the full 128 partitions (wA on 0:64, wB on 64:128 via PE column
tiling), so the ScalarE softplus (Exp+Ln, one shared LUT table) runs at
full lane utilization.  Matmul operand paths are bf16 (PSUM accumulation
stays fp32); padding edges have zeroed gathered features so they add 0.

Device pipeline per window pair (T=5 tiles of 128 edges per window):
  rbf_T pair  --DMA-->  SBUF [128, 2*EW] bf16
  MM1  : psum[0:64]=W1.T@rbfA, psum[64:128]=W1.T@rbfB (col-tiled), x2 chunks
  ACT  : ex = Exp(0.5*psum + 0.5*b1);  h1 = Ln(ex + 1)   [128, 320] each
  per window:
    MM2  : psum2[128, T*64] = h1_half.T @ (2*W2)       (K=64, T tiles)
    DVE  : msg = psum2 * gathered                       (bf16 out)
    DVE  : S_T = is_equal(dst_slot, iota)               [128, T*128] bf16
    MMr  : pso[:, 0:128]   += msg_t.T @ S_T_t           (T tiles)
           pso[:, 128:256] += gath_t.T @ S_T_t          (b2 path)
    DVE  : stg = pso[:, 128:256]*b2 + pso[:, 0:128]     (fused)
  DMA out every OGRP windows.
"""

import os
import numpy as np

N_NODES = 100_000
RBF = 128
DIM = 64
CORES = 8
W_TOTAL = 784              # 128-node windows; 784*128 = 100352 >= N_NODES
WPC = W_TOTAL // CORES     # 98 windows per core
NODES_CAP = W_TOTAL * 128  # 100352


# ----------------------------------------------------------------------------
# Host-side preprocessing
# ----------------------------------------------------------------------------

def _prepare(rbf, new_node, src, dst, edge_mask, W1, b1, W2, b2,
             bf16_data=True, bf16_gath=True):
    import ml_dtypes
    bf = ml_dtypes.bfloat16

    mask = np.asarray(edge_mask).astype(bool)
    kept = np.nonzero(mask)[0]
    src_k = np.asarray(src)[kept].astype(np.int64)
    dst_k = np.asarray(dst)[kept].astype(np.int64)
    Ek = len(kept)

    # --- node -> (window, slot) assignment, balanced by in-degree ---
    deg = np.bincount(dst_k, minlength=NODES_CAP)
    order = np.argsort(-deg, kind="stable")
    node_win = np.empty(NODES_CAP, np.int64)
    node_slot = np.empty(NODES_CAP, np.int64)
    fwd = np.arange(W_TOTAL)
    bwd = fwd[::-1]
    for r in range(128):  # serpentine deal: round r gives each window 1 node
        idx = order[r * W_TOTAL:(r + 1) * W_TOTAL]
        node_win[idx] = fwd if (r % 2 == 0) else bwd
        node_slot[idx] = r

    ewin = node_win[dst_k]
    loads = np.bincount(ewin, minlength=W_TOTAL)
    T = max(2, int(np.ceil(loads.max() / 128)))  # tiles of 128 edges per window
    EW = T * 128
    EPAD = W_TOTAL * EW

    # --- edge placement: group edges by window, pad windows to EW ---
    order_e = np.argsort(ewin, kind="stable")
    ewin_s = ewin[order_e]
    cum = np.concatenate([[0], np.cumsum(loads)])
    pos = (np.arange(Ek) - cum[ewin_s]) + ewin_s * EW  # padded slot per edge

    dstoff_full = np.zeros(EPAD, np.float32)
    dstoff_full[pos] = node_slot[dst_k[order_e]]

    rbf_dt = bf if bf16_data else np.float32
    rbf_full = np.zeros((EPAD, RBF), rbf_dt)
    rbf_full[pos] = np.asarray(rbf, np.float32)[kept[order_e]].astype(rbf_dt)

    # Host-staged gather of source-node features into padded edge order.
    # (Padding/masked slots stay zero, which also implements edge masking.)
    gath_dt = bf if bf16_gath else np.float32
    gath_full = np.zeros((EPAD, DIM), gath_dt)
    gath_full[pos] = np.asarray(new_node, np.float32)[src_k[order_e]].astype(gath_dt)

    # --- per-core input tensors ---
    NT = WPC * T
    wdt = bf if bf16_data else np.float32
    rbft_c = np.ascontiguousarray(
        rbf_full.reshape(CORES, WPC * EW, RBF).transpose(0, 2, 1))
    dstof_c = np.ascontiguousarray(
        dstoff_full.reshape(CORES, WPC, T, 128).transpose(0, 3, 1, 2)
        .reshape(CORES, 128, NT).astype(wdt))
    gath_c = np.ascontiguousarray(
        gath_full.reshape(CORES, WPC, T, 128, DIM).transpose(0, 3, 1, 2, 4)
        .reshape(CORES, 128, NT * DIM))

    w1 = np.ascontiguousarray(np.asarray(W1, np.float32).astype(wdt))
    w2d = np.ascontiguousarray(np.vstack(
        [2.0 * np.asarray(W2, np.float32)] * 2).astype(wdt))       # [128, 64]
    b1h2 = np.ascontiguousarray(np.tile(
        0.5 * np.asarray(b1, np.float32)[:, None], (2, 1)))        # [128, 1]
    b2c = np.ascontiguousarray(np.asarray(b2, np.float32)[:, None])  # [64, 1]
    iota = np.ascontiguousarray(np.broadcast_to(
        np.arange(128, dtype=np.float32), (128, 128)).astype(wdt))

    in_maps = []
    for c in range(CORES):
        in_maps.append({
            "rbft": rbft_c[c],
            "gath": gath_c[c],
            "dstof": dstof_c[c],
            "w1": w1,
            "w2d": w2d,
            "b1h2": b1h2,
            "b2c": b2c,
            "iota": iota,
        })
    return T, in_maps, node_win, node_slot


# ----------------------------------------------------------------------------
# Device program
# ----------------------------------------------------------------------------

def _patch_act_tables():
    """Force the Exp/Ln activation-table chooser onto the one table that
    contains both (natural_log_exp_and_others), so the ACT engine loads a
    table once instead of flip-flopping between exp- and ln-only tables
    (1283ns per reload). Keys/order preserved so act_func_set_id stays valid."""
    import functools
    import concourse.bacc as bacc
    import concourse.hw_specs as hw_specs
    if getattr(bacc, "_act_tables_patched", False):
        return
    real = hw_specs.get_activation_tables

    @functools.cache
    def only_shared(arch):
        tabs = dict(real(arch))
        keep = "natural_log_exp_and_others"
        return {k: (v if k == keep else set()) for k, v in tabs.items()}

    bacc.get_activation_tables = only_shared
    bacc._act_tables_patched = True


def _build(T, opt=None):
    import dataclasses as _dc
    import concourse.bass as bass
    import concourse.bacc as bacc
    import concourse.mybir as mybir
    import concourse.tile as tile
    _patch_act_tables()

    EW = T * 128
    NT = WPC * T
    ECORE = WPC * EW
    HALF = EW // 2
    NPAIR = WPC // 2

    opt = dict(opt or {})
    BF = opt.get("bf16_data", True)
    BFG = opt.get("bf16_gath", True)
    H1N = opt.get("h1n", 2)
    OGRP = opt.get("ogrp", 4)        # windows per output DMA / gather load
    IOB = opt.get("iob", 3)
    WKB = opt.get("wkb", 3)
    PS1B = opt.get("ps1b", 4)
    PS2B = opt.get("ps2b", 2)
    PSOB = opt.get("psob", 2)

    fp32 = mybir.dt.float32
    bf16 = mybir.dt.bfloat16
    ddt = bf16 if BF else fp32
    gdt = bf16 if BFG else fp32

    nc = bacc.Bacc("TRN2", target_bir_lowering=False, debug=False)

    rbft = nc.dram_tensor("rbft", [128, ECORE], ddt, kind="ExternalInput")
    gath = nc.dram_tensor("gath", [128, NT * DIM], gdt, kind="ExternalInput")
    dstof = nc.dram_tensor("dstof", [128, NT], ddt, kind="ExternalInput")
    w1 = nc.dram_tensor("w1", [RBF, DIM], ddt, kind="ExternalInput")
    w2d = nc.dram_tensor("w2d", [128, DIM], ddt, kind="ExternalInput")
    b1h2 = nc.dram_tensor("b1h2", [128, 1], fp32, kind="ExternalInput")
    b2c = nc.dram_tensor("b2c", [DIM, 1], fp32, kind="ExternalInput")
    iota = nc.dram_tensor("iota", [128, 128], ddt, kind="ExternalInput")
    out = nc.dram_tensor("out", [DIM, WPC * 128], fp32, kind="ExternalOutput")

    EXP = mybir.ActivationFunctionType.Exp
    LN = mybir.ActivationFunctionType.Ln
    MUL = mybir.AluOpType.mult
    ADD = mybir.AluOpType.add
    EQ = mybir.AluOpType.is_equal

    with tile.TileContext(nc) as tc:
        with (
            tc.tile_pool(name="persist", bufs=1) as pp,
            tc.tile_pool(name="io", bufs=IOB) as io,
            tc.tile_pool(name="wk", bufs=WKB) as wk,
            tc.tile_pool(name="stgp", bufs=2) as stgp,
            tc.tile_pool(name="ps1", bufs=PS1B, space="PSUM") as ps1p,
            tc.tile_pool(name="ps2", bufs=PS2B, space="PSUM") as ps2p,
            tc.tile_pool(name="pso", bufs=PSOB, space="PSUM") as psop,
        ):
            w1_sb = pp.tile([RBF, DIM], ddt)
            nc.sync.dma_start(w1_sb[:], w1[:])
            w2d_sb = pp.tile([128, DIM], ddt)
            nc.sync.dma_start(w2d_sb[:], w2d[:])
            b1h2_sb = pp.tile([128, 1], fp32)
            nc.sync.dma_start(b1h2_sb[:], b1h2[:])
            b2c_sb = pp.tile([DIM, 1], fp32)
            nc.sync.dma_start(b2c_sb[:], b2c[:])
            iota_sb = pp.tile([128, 128], ddt)
            nc.sync.dma_start(iota_sb[:], iota[:])
            dstof_sb = pp.tile([128, NT], ddt)
            nc.sync.dma_start(dstof_sb[:], dstof[:])

            h1s = [pp.tile([128, EW], ddt, tag=f"h1s{i}", name=f"h1s{i}")
                   for i in range(H1N)]

            _ia = iota_sb[:]
            iota_b = _dc.replace(_ia, ap=[_ia.ap[0], [0, T], _ia.ap[1]])

            stg = None
            gat4 = None
            for q in range(NPAIR):
                rbfp = io.tile([128, 2 * EW], ddt, tag="rbfp")
                nc.sync.dma_start(rbfp[:], rbft[:, q * 2 * EW:(q + 1) * 2 * EW])

                h1 = h1s[q % H1N]
                for c in range(2):
                    ps1 = ps1p.tile([128, HALF], fp32, tag="mm1")
                    nc.tensor.matmul(
                        ps1[0:DIM, :], w1_sb[:],
                        rbfp[:, c * HALF:(c + 1) * HALF],
                        start=True, stop=True)
                    nc.tensor.matmul(
                        ps1[DIM:128, :], w1_sb[:],
                        rbfp[:, EW + c * HALF:EW + (c + 1) * HALF],
                        start=True, stop=True, tile_position=(0, 64))
                    # softplus(y) = ln(1 + exp(y)), y = 0.5*x + 0.5*b1
                    ex = wk.tile([128, HALF], fp32, tag="ex")
                    nc.scalar.activation(
                        ex[:], ps1[:], EXP, bias=b1h2_sb[:], scale=0.5)
                    nc.scalar.activation(
                        h1[:, c * HALF:(c + 1) * HALF], ex[:], LN, bias=1.0)

                for sub in range(2):
                    w = 2 * q + sub
                    g = w % OGRP
                    if g == 0:
                        stg = stgp.tile([DIM, OGRP * 128], fp32, tag="stg",
                                        name="stg")
                        gat4 = io.tile([128, OGRP * T * DIM], gdt, tag="gat",
                                       name="gat4")
                        ng = min(OGRP, WPC - w)
                        nc.sync.dma_start(
                            gat4[:, :ng * T * DIM],
                            gath[:, w * T * DIM:(w + ng) * T * DIM])

                    base = sub * DIM
                    ps2 = ps2p.tile([128, T * DIM], fp32, tag="mm2")
                    for t in range(T):
                        nc.tensor.matmul(
                            ps2[:, t * DIM:(t + 1) * DIM],
                            h1[base:base + DIM, t * 128:(t + 1) * 128],
                            w2d_sb[base:base + DIM, :],
                            start=True, stop=True)

                    st = wk.tile([128, T * 128], ddt, tag="st")
                    nc.vector.tensor_tensor(
                        out=st[:].rearrange("p (t n) -> p t n", t=T),
                        in0=dstof_sb[:, w * T:(w + 1) * T].to_broadcast(
                            [128, T, 128]),
                        in1=iota_b,
                        op=EQ)

                    msg = wk.tile([128, T * DIM], ddt, tag="msg")
                    nc.vector.tensor_tensor(
                        out=msg[:], in0=ps2[:],
                        in1=gat4[:, g * T * DIM:(g + 1) * T * DIM], op=MUL)

                    # NOTE: the two accumulation groups must not interleave
                    # within one PSUM bank (HW corrupts the first group).
                    pso = psop.tile([DIM, 256], fp32, tag="out")
                    for t in range(T):
                        nc.tensor.matmul(
                            pso[:, 0:128], msg[:, t * DIM:(t + 1) * DIM],
                            st[:, t * 128:(t + 1) * 128],
                            start=(t == 0), stop=(t == T - 1))
                    for t in range(T):
                        nc.tensor.matmul(
                            pso[:, 128:256],
                            gat4[:, (g * T + t) * DIM:(g * T + t + 1) * DIM],
                            st[:, t * 128:(t + 1) * 128],
                            start=(t == 0), stop=(t == T - 1))

                    # stg = pso_g * b2 + pso_msg  (2 ops: DVE has 1 PSUM port)
                    gb2 = wk.tile([DIM, 128], fp32, tag="gb2")
                    nc.vector.tensor_scalar_mul(gb2[:], pso[:, 128:256], b2c_sb[:])
                    nc.vector.tensor_tensor(
                        out=stg[:, g * 128:(g + 1) * 128],
                        in0=pso[:, 0:128], in1=gb2[:], op=ADD)

                    if g == OGRP - 1 or w == WPC - 1:
                        w0 = w - g
                        nc.sync.dma_start(
                            out[:, w0 * 128:(w + 1) * 128],
                            stg[:, :(g + 1) * 128])

    nc.compile()
    return nc


_CACHE = {}


def _get_nc(T, opt=None):
    key = (T, tuple(sorted((opt or {}).items())))
    if key not in _CACHE:
        _CACHE[key] = _build(T, opt)
    return _CACHE[key]


# ----------------------------------------------------------------------------
# Entry point
# ----------------------------------------------------------------------------

def kernel(rbf, new_node, src, dst, edge_mask, W1, b1, W2, b2):
    T, in_maps, node_win, node_slot = _prepare(
        rbf, new_node, src, dst, edge_mask, W1, b1, W2, b2)
    nc = _get_nc(T)

    if os.environ.get("CFCONV_SIM"):
        outs = [_emulate_core(in_maps[c]) for c in range(CORES)]
    else:
        from concourse.bass_utils import run_bass_kernel_spmd
        res = run_bass_kernel_spmd(nc, in_maps, core_ids=list(range(CORES)))
        outs = [r["out"] for r in res.results]

    full = np.concatenate(outs, axis=1)  # [64, 100352]
    col = node_win[:N_NODES] * 128 + node_slot[:N_NODES]
    result = np.ascontiguousarray(full[:, col].T.astype(np.float32))
    return result


def _emulate_core(in_map):
    """Numpy emulation of the device program for one core (debug only)."""
    f32 = np.float32
    rbft = in_map["rbft"].astype(f32)
    gath = in_map["gath"].astype(f32)
    dstof = in_map["dstof"].astype(f32)
    w1 = in_map["w1"].astype(f32)
    w2d = in_map["w2d"].astype(f32)
    b1h2 = in_map["b1h2"].astype(f32)
    b2c = in_map["b2c"].astype(f32)
    T = dstof.shape[1] // WPC
    EW = T * 128
    out = np.zeros((DIM, WPC * 128), f32)
    for w in range(WPC):
        rb = rbft[:, w * EW:(w + 1) * EW]
        h1 = np.log1p(np.exp((w1.T @ rb) * 0.5 + b1h2[:DIM]))
        dof = dstof[:, w * T:(w + 1) * T]
        ga = gath[:, w * T * DIM:(w + 1) * T * DIM].reshape(128, T, DIM)
        accm = np.zeros((DIM, 128), f32)
        accg = np.zeros((DIM, 128), f32)
        for t in range(T):
            h2 = h1[:, t * 128:(t + 1) * 128].T @ w2d[:DIM]
            msg = h2 * ga[:, t]
            stt = (dof[:, t:t + 1] == np.arange(128)[None, :]).astype(f32)
            accm += msg.T @ stt
            accg += ga[:, t].T @ stt
        out[:, w * 128:(w + 1) * 128] = accm + b2c * accg
    return out


# revision 13
# speedup vs baseline: 1.1594x; 1.1594x over previous
"""CFConvCluster Trainium2 kernel (8 NeuronCores, SPMD, no collectives).

Strategy
--------
The reference computes, per edge e:  msg_e = mask_e * new_node[src_e] * MLP(rbf_e)
and scatter-sums msg into dst nodes.  Exact algebraic restructurings:

1. Masked edges contribute exactly zero -> dropped up front (E: 1.6M -> ~449k).
2. Nodes are relabeled (host permutation) into 784 balanced 128-node
   "windows"; edges grouped by the window of their dst.  The segment-sum
   for a window is a matmul with a one-hot selection matrix
   S_T[e, n] = (dst_e == slot n), accumulated over the window's edge
   tiles in PSUM.  Output ranges are disjoint across cores -> no
   all-reduce; the host concatenates and un-permutes rows.
3. b2 folds via linearity: sum S*(h2+b2)*g = sum S*(h2*g) + b2*sum S*g,
   computed as twin accumulated matmuls sharing one PSUM tile, combined
   by a single fused DVE op.

Windows are processed in PAIRS: the edge MLP for window pair (wA, wB)
runs on the full 128 partitions (wA on 0:64, wB on 64:128 via PE column
tiling), so the ScalarE softplus (Exp+Ln, one shared LUT table) runs at
full lane utilization.  Matmul operand paths are bf16 (PSUM accumulation
stays fp32); padding edges have zeroed gathered features so they add 0.

Device pipeline per window pair (T=5 tiles of 128 edges per window):
  rbf_T pair  --DMA-->  SBUF [128, 2*EW] bf16
  MM1  : psum[0:64]=W1.T@rbfA, psum[64:128]=W1.T@rbfB (col-tiled), x2 chunks
  ACT  : ex = Exp(0.5*psum + 0.5*b1);  h1 = Ln(ex + 1)   [128, 320] each
  per window:
    MM2  : psum2[128, T*64] = h1_half.T @ (2*W2)       (K=64, T tiles)
    DVE  : msg = psum2 * gathered                       (bf16 out)
    DVE  : S_T = is_equal(dst_slot, iota)               [128, T*128] bf16
    MMr  : pso[:, 0:128]   += msg_t.T @ S_T_t           (T tiles)
           pso[:, 128:256] += gath_t.T @ S_T_t          (b2 path)
    DVE  : stg = pso[:, 128:256]*b2 + pso[:, 0:128]     (fused)
  DMA out every OGRP windows.
"""

import os
import numpy as np

N_NODES = 100_000
RBF = 128
DIM = 64
CORES = 8
W_TOTAL = 784              # 128-node windows; 784*128 = 100352 >= N_NODES
WPC = W_TOTAL // CORES     # 98 windows per core
NODES_CAP = W_TOTAL * 128  # 100352


# ----------------------------------------------------------------------------
# Host-side preprocessing
# ----------------------------------------------------------------------------

def _prepare(rbf, new_node, src, dst, edge_mask, W1, b1, W2, b2,
             bf16_data=True, bf16_gath=True):
    import ml_dtypes
    bf = ml_dtypes.bfloat16

    mask = np.asarray(edge_mask).astype(bool)
    kept = np.nonzero(mask)[0]
    src_k = np.asarray(src)[kept].astype(np.int64)
    dst_k = np.asarray(dst)[kept].astype(np.int64)
    Ek = len(kept)

    # --- node -> (window, slot) assignment, balanced by in-degree ---
    deg = np.bincount(dst_k, minlength=NODES_CAP)
    order = np.argsort(-deg, kind="stable")
    node_win = np.empty(NODES_CAP, np.int64)
    node_slot = np.empty(NODES_CAP, np.int64)
    fwd = np.arange(W_TOTAL)
    bwd = fwd[::-1]
    for r in range(128):  # serpentine deal: round r gives each window 1 node
        idx = order[r * W_TOTAL:(r + 1) * W_TOTAL]
        node_win[idx] = fwd if (r % 2 == 0) else bwd
        node_slot[idx] = r

    ewin = node_win[dst_k]
    loads = np.bincount(ewin, minlength=W_TOTAL)
    T = max(2, int(np.ceil(loads.max() / 128)))  # tiles of 128 edges per window
    EW = T * 128
    EPAD = W_TOTAL * EW

    # --- edge placement: group edges by window, pad windows to EW ---
    order_e = np.argsort(ewin, kind="stable")
    ewin_s = ewin[order_e]
    cum = np.concatenate([[0], np.cumsum(loads)])
    pos = (np.arange(Ek) - cum[ewin_s]) + ewin_s * EW  # padded slot per edge

    dstoff_full = np.zeros(EPAD, np.float32)
    dstoff_full[pos] = node_slot[dst_k[order_e]]

    rbf_dt = bf if bf16_data else np.float32
    rbf_full = np.zeros((EPAD, RBF), rbf_dt)
    rbf_full[pos] = np.asarray(rbf, np.float32)[kept[order_e]].astype(rbf_dt)

    # Host-staged gather of source-node features into padded edge order.
    # (Padding/masked slots stay zero, which also implements edge masking.)
    gath_dt = bf if bf16_gath else np.float32
    gath_full = np.zeros((EPAD, DIM), gath_dt)
    gath_full[pos] = np.asarray(new_node, np.float32)[src_k[order_e]].astype(gath_dt)

    # --- per-core input tensors ---
    NT = WPC * T
    wdt = bf if bf16_data else np.float32
    rbft_c = np.ascontiguousarray(
        rbf_full.reshape(CORES, WPC * EW, RBF).transpose(0, 2, 1))
    dstof_c = np.ascontiguousarray(
        dstoff_full.reshape(CORES, WPC, T, 128).transpose(0, 3, 1, 2)
        .reshape(CORES, 128, NT).astype(wdt))
    gath_c = np.ascontiguousarray(
        gath_full.reshape(CORES, WPC, T, 128, DIM).transpose(0, 3, 1, 2, 4)
        .reshape(CORES, 128, NT * DIM))

    w1 = np.ascontiguousarray(np.asarray(W1, np.float32).astype(wdt))
    w2d = np.ascontiguousarray(np.vstack(
        [2.0 * np.asarray(W2, np.float32)] * 2).astype(wdt))       # [128, 64]
    b1h2 = np.ascontiguousarray(np.tile(
        0.5 * np.asarray(b1, np.float32)[:, None], (2, 1)))        # [128, 1]
    b2c = np.ascontiguousarray(np.asarray(b2, np.float32)[:, None])  # [64, 1]
    # iota_rep[p, n*T + j] = n  (fully packed operand for the S_T build)
    iota = np.ascontiguousarray(np.repeat(
        np.arange(128, dtype=np.float32), T)[None, :].repeat(128, 0).astype(wdt))

    in_maps = []
    for c in range(CORES):
        in_maps.append({
            "rbft": rbft_c[c],
            "gath": gath_c[c],
            "dstof": dstof_c[c],
            "w1": w1,
            "w2d": w2d,
            "b1h2": b1h2,
            "b2c": b2c,
            "iota": iota,
        })
    return T, in_maps, node_win, node_slot


# ----------------------------------------------------------------------------
# Device program
# ----------------------------------------------------------------------------

def _patch_act_tables():
    """Force the Exp/Ln activation-table chooser onto the one table that
    contains both (natural_log_exp_and_others), so the ACT engine loads a
    table once instead of flip-flopping between exp- and ln-only tables
    (1283ns per reload). Keys/order preserved so act_func_set_id stays valid."""
    import functools
    import concourse.bacc as bacc
    import concourse.hw_specs as hw_specs
    if getattr(bacc, "_act_tables_patched", False):
        return
    real = hw_specs.get_activation_tables

    @functools.cache
    def only_shared(arch):
        tabs = dict(real(arch))
        keep = "natural_log_exp_and_others"
        return {k: (v if k == keep else set()) for k, v in tabs.items()}

    bacc.get_activation_tables = only_shared
    bacc._act_tables_patched = True


def _build(T, opt=None):
    import dataclasses as _dc
    import concourse.bass as bass
    import concourse.bacc as bacc
    import concourse.mybir as mybir
    import concourse.tile as tile
    _patch_act_tables()

    EW = T * 128
    NT = WPC * T
    ECORE = WPC * EW
    HALF = EW // 2
    NPAIR = WPC // 2

    opt = dict(opt or {})
    BF = opt.get("bf16_data", True)
    BFG = opt.get("bf16_gath", True)
    H1N = opt.get("h1n", 2)
    OGRP = opt.get("ogrp", 4)        # windows per output DMA / gather load
    IOB = opt.get("iob", 3)
    WKB = opt.get("wkb", 3)
    PS1B = opt.get("ps1b", 4)
    PS2B = opt.get("ps2b", 2)
    PSOB = opt.get("psob", 2)

    fp32 = mybir.dt.float32
    bf16 = mybir.dt.bfloat16
    ddt = bf16 if BF else fp32
    gdt = bf16 if BFG else fp32

    nc = bacc.Bacc("TRN2", target_bir_lowering=False, debug=False)

    rbft = nc.dram_tensor("rbft", [128, ECORE], ddt, kind="ExternalInput")
    gath = nc.dram_tensor("gath", [128, NT * DIM], gdt, kind="ExternalInput")
    dstof = nc.dram_tensor("dstof", [128, NT], ddt, kind="ExternalInput")
    w1 = nc.dram_tensor("w1", [RBF, DIM], ddt, kind="ExternalInput")
    w2d = nc.dram_tensor("w2d", [128, DIM], ddt, kind="ExternalInput")
    b1h2 = nc.dram_tensor("b1h2", [128, 1], fp32, kind="ExternalInput")
    b2c = nc.dram_tensor("b2c", [DIM, 1], fp32, kind="ExternalInput")
    iota = nc.dram_tensor("iota", [128, 128 * T], ddt, kind="ExternalInput")
    out = nc.dram_tensor("out", [DIM, WPC * 128], fp32, kind="ExternalOutput")

    EXP = mybir.ActivationFunctionType.Exp
    LN = mybir.ActivationFunctionType.Ln
    CP = mybir.ActivationFunctionType.Copy
    MUL = mybir.AluOpType.mult
    ADD = mybir.AluOpType.add
    EQ = mybir.AluOpType.is_equal

    with tile.TileContext(nc) as tc:
        with (
            tc.tile_pool(name="persist", bufs=1) as pp,
            tc.tile_pool(name="io", bufs=IOB) as io,
            tc.tile_pool(name="wk", bufs=WKB) as wk,
            tc.tile_pool(name="stgp", bufs=2) as stgp,
            tc.tile_pool(name="ps1", bufs=PS1B, space="PSUM") as ps1p,
            tc.tile_pool(name="ps2", bufs=PS2B, space="PSUM") as ps2p,
            tc.tile_pool(name="pso", bufs=PSOB, space="PSUM") as psop,
        ):
            w1_sb = pp.tile([RBF, DIM], ddt)
            nc.sync.dma_start(w1_sb[:], w1[:])
            w2d_sb = pp.tile([128, DIM], ddt)
            nc.sync.dma_start(w2d_sb[:], w2d[:])
            b1h2_sb = pp.tile([128, 1], fp32)
            nc.sync.dma_start(b1h2_sb[:], b1h2[:])
            b2c_sb = pp.tile([DIM, 1], fp32)
            nc.sync.dma_start(b2c_sb[:], b2c[:])
            iota_sb = pp.tile([128, 128 * T], ddt)
            nc.sync.dma_start(iota_sb[:], iota[:])
            dstof_sb = pp.tile([128, NT], ddt)
            nc.sync.dma_start(dstof_sb[:], dstof[:])

            h1s = [pp.tile([128, EW], ddt, tag=f"h1s{i}", name=f"h1s{i}")
                   for i in range(H1N)]

            stg = None
            gat4 = None
            for q in range(NPAIR):
                rbfp = io.tile([128, 2 * EW], ddt, tag="rbfp")
                nc.sync.dma_start(rbfp[:], rbft[:, q * 2 * EW:(q + 1) * 2 * EW])

                h1 = h1s[q % H1N]
                for c in range(2):
                    ps1 = ps1p.tile([128, HALF], fp32, tag="mm1")
                    nc.tensor.matmul(
                        ps1[0:DIM, :], w1_sb[:],
                        rbfp[:, c * HALF:(c + 1) * HALF],
                        start=True, stop=True)
                    nc.tensor.matmul(
                        ps1[DIM:128, :], w1_sb[:],
                        rbfp[:, EW + c * HALF:EW + (c + 1) * HALF],
                        start=True, stop=True, tile_position=(0, 64))
                    # softplus(y) = ln(1 + exp(y)), y = 0.5*x + 0.5*b1
                    ex = wk.tile([128, HALF], fp32, tag="ex")
                    nc.scalar.activation(
                        ex[:], ps1[:], EXP, bias=b1h2_sb[:], scale=0.5)
                    nc.scalar.activation(
                        h1[:, c * HALF:(c + 1) * HALF], ex[:], LN, bias=1.0)

                for sub in range(2):
                    w = 2 * q + sub
                    g = w % OGRP
                    if g == 0:
                        stg = stgp.tile([DIM, OGRP * 128], fp32, tag="stg",
                                        name="stg")
                        gat4 = io.tile([128, OGRP * T * DIM], gdt, tag="gat",
                                       name="gat4")
                        ng = min(OGRP, WPC - w)
                        nc.sync.dma_start(
                            gat4[:, :ng * T * DIM],
                            gath[:, w * T * DIM:(w + ng) * T * DIM])

                    base = sub * DIM
                    ps2 = ps2p.tile([128, T * DIM], fp32, tag="mm2")
                    for t in range(T):
                        nc.tensor.matmul(
                            ps2[:, t * DIM:(t + 1) * DIM],
                            h1[base:base + DIM, t * 128:(t + 1) * 128],
                            w2d_sb[base:base + DIM, :],
                            start=True, stop=True)

                    # S_T2[p, n, t] = (dst_slot[p, t] == n); fully packed
                    # last dims on every operand -> DVE 2x/4x perf mode.
                    st = wk.tile([128, 128 * T], ddt, tag="st")
                    _dv = dstof_sb[:, w * T:(w + 1) * T]
                    nc.vector.tensor_tensor(
                        out=st[:].rearrange("p (n t) -> p n t", t=T),
                        in0=_dc.replace(_dv, ap=[_dv.ap[0], [0, 128], [1, T]]),
                        in1=iota_sb[:].rearrange("p (n t) -> p n t", t=T),
                        op=EQ)

                    msg = wk.tile([128, T * DIM], ddt, tag="msg")
                    nc.vector.tensor_tensor(
                        out=msg[:], in0=ps2[:],
                        in1=gat4[:, g * T * DIM:(g + 1) * T * DIM], op=MUL)

                    # NOTE: the two accumulation groups must not interleave
                    # within one PSUM bank (HW corrupts the first group).
                    pso = psop.tile([DIM, 256], fp32, tag="out")
                    _st = st[:]
                    st_ts = [_dc.replace(_st, offset=_st.offset + t,
                                         ap=[_st.ap[0], [T, 128]])
                             for t in range(T)]
                    for t in range(T):
                        nc.tensor.matmul(
                            pso[:, 0:128], msg[:, t * DIM:(t + 1) * DIM],
                            st_ts[t], start=(t == 0), stop=(t == T - 1))
                    for t in range(T):
                        nc.tensor.matmul(
                            pso[:, 128:256],
                            gat4[:, (g * T + t) * DIM:(g * T + t + 1) * DIM],
                            st_ts[t], start=(t == 0), stop=(t == T - 1))

                    # stg = pso_g * b2 + pso_msg  (2 ops: 1 PSUM port each;
                    # the scale-by-b2 runs on ACT to offload DVE)
                    gb2 = wk.tile([DIM, 128], fp32, tag="gb2")
                    nc.scalar.activation(gb2[:], pso[:, 128:256], CP,
                                         bias=0.0, scale=b2c_sb[:])
                    nc.vector.tensor_tensor(
                        out=stg[:, g * 128:(g + 1) * 128],
                        in0=pso[:, 0:128], in1=gb2[:], op=ADD)

                    if g == OGRP - 1 or w == WPC - 1:
                        w0 = w - g
                        nc.sync.dma_start(
                            out[:, w0 * 128:(w + 1) * 128],
                            stg[:, :(g + 1) * 128])

    nc.compile()
    return nc


_CACHE = {}


def _get_nc(T, opt=None):
    key = (T, tuple(sorted((opt or {}).items())))
    if key not in _CACHE:
        _CACHE[key] = _build(T, opt)
    return _CACHE[key]


# ----------------------------------------------------------------------------
# Entry point
# ----------------------------------------------------------------------------

def kernel(rbf, new_node, src, dst, edge_mask, W1, b1, W2, b2):
    T, in_maps, node_win, node_slot = _prepare(
        rbf, new_node, src, dst, edge_mask, W1, b1, W2, b2)
    nc = _get_nc(T)

    if os.environ.get("CFCONV_SIM"):
        outs = [_emulate_core(in_maps[c]) for c in range(CORES)]
    else:
        from concourse.bass_utils import run_bass_kernel_spmd
        res = run_bass_kernel_spmd(nc, in_maps, core_ids=list(range(CORES)))
        outs = [r["out"] for r in res.results]

    full = np.concatenate(outs, axis=1)  # [64, 100352]
    col = node_win[:N_NODES] * 128 + node_slot[:N_NODES]
    result = np.ascontiguousarray(full[:, col].T.astype(np.float32))
    return result


def _emulate_core(in_map):
    """Numpy emulation of the device program for one core (debug only)."""
    f32 = np.float32
    rbft = in_map["rbft"].astype(f32)
    gath = in_map["gath"].astype(f32)
    dstof = in_map["dstof"].astype(f32)
    w1 = in_map["w1"].astype(f32)
    w2d = in_map["w2d"].astype(f32)
    b1h2 = in_map["b1h2"].astype(f32)
    b2c = in_map["b2c"].astype(f32)
    T = dstof.shape[1] // WPC
    EW = T * 128
    out = np.zeros((DIM, WPC * 128), f32)
    for w in range(WPC):
        rb = rbft[:, w * EW:(w + 1) * EW]
        h1 = np.log1p(np.exp((w1.T @ rb) * 0.5 + b1h2[:DIM]))
        dof = dstof[:, w * T:(w + 1) * T]
        ga = gath[:, w * T * DIM:(w + 1) * T * DIM].reshape(128, T, DIM)
        accm = np.zeros((DIM, 128), f32)
        accg = np.zeros((DIM, 128), f32)
        for t in range(T):
            h2 = h1[:, t * 128:(t + 1) * 128].T @ w2d[:DIM]
            msg = h2 * ga[:, t]
            stt = (dof[:, t:t + 1] == np.arange(128)[None, :]).astype(f32)
            accm += msg.T @ stt
            accg += ga[:, t].T @ stt
        out[:, w * 128:(w + 1) * 128] = accm + b2c * accg
    return out
